# revision 14
# baseline (speedup 1.0000x reference)
"""MultiHeadAttention kernel for 8x TRN2 NeuronCores.

The reference module's einsum reduces the attention tensor over BOTH the
query and key axes (attn_mass = sum_{q,k} softmax(logits)_k), and softmax
rows sum to 1, so attn_mass == Lq exactly for every (batch, head).  The
whole computation collapses to a single dense GEMM after folding the
(block-diagonal) per-head V-projection into the output projection:

    out = V_flat @ W_eff + b_eff          (4096 x 1024) @ (1024 x 1024)
    W_eff[h*hd+a, n] = Lq * sum_b Wv[b, a] * Wo[n, h*hd+b]
    b_eff[n]         = Lq * sum_{h,b} Wo[n, h*hd+b] * bv[b] + bo[n]

Row-sharded across 8 cores (512 rows each), computed TRANSPOSED so the
bias is a per-partition scalar folded into the PSUM eviction.  All
operands stream as bf16 (tolerance is 2e-2; bf16 end-to-end lands at
~2.6e-3), which halves HBM traffic vs fp32 and runs the PE at 1 row/cycle.

Schedule (per core), driven by trace analysis:
  * every HWDGE queue has ~1.4us of DGE start latency + 0.9us completion
    semaphore propagation, and each dma_start burns ~0.65us of the issuing
    engine's sequencer -- so the stream is a handful of large DMAs spread
    over THREE queues (sync / vector / scalar) for dispatch parallelism;
  * a tiny fast-start DMA [W0k0 | X0] (160 KB) leads the sync queue and
    unblocks the first real matmul ~4us in; bf16 junk matmuls on memset
    data keep the PE busy (HAM clock gate + p-state ramp) until then;
  * X is fronted across sync+vector queues (bank 0's k-sweep is gated by
    X arrival), W banks 1-7 follow; banks retire in order afterwards,
    purely compute-bound, so evictions + output DMAs overlap compute;
  * the last bank's eviction is split so its output DMA overlaps the
    second half's tensor_scalar_add.
"""

import numpy as np
import ml_dtypes

import concourse.bass as bass
import concourse.bacc as bacc
import concourse.mybir as mybir
from concourse.tile import TileContext
from concourse.bass_utils import run_bass_kernel_spmd

N_CORES = 8
E = 1024            # embed dim == d_model
H, HD = 16, 64      # heads, head dim
ROWS = 4096         # N * L = 2 * 2048
RPC = ROWS // N_CORES   # rows per core = 512
P = 128             # SBUF partitions
KT = E // P         # 8 contraction slabs
JT = E // P         # 8 output-column banks

# -- tuning knobs ------------------------------------------------------
N_WARM = 6          # junk matmuls before the first real matmul
JF = 512            # junk matmul free dim
JF2 = 256           # filler junk free dim (inside the X-gated phase)

BF16 = ml_dtypes.bfloat16

_NC_CACHE = {}
LAST_RESULTS = None  # BassKernelResults of the most recent device run


def _build():
    f32 = mybir.dt.float32
    bf = mybir.dt.bfloat16
    nc = bacc.Bacc(None, target_bir_lowering=False)

    # hd = [W0k0 | X0] fast-start block; wc0r = W0 k=1..7; xsp = X slabs
    # 1..7 packed; wcp = W banks 1..7 packed; bias per-bank per-partition.
    hd = nc.declare_dram_parameter("hd", [P, P + RPC], bf, isOutput=False)
    wc0r = nc.declare_dram_parameter("wc0r", [P, (KT - 1) * P], bf, isOutput=False)
    xsp = nc.declare_dram_parameter("xsp", [P, (KT - 1) * RPC], bf, isOutput=False)
    wcp = nc.declare_dram_parameter("wcp", [P, (JT - 1) * E], bf, isOutput=False)
    bias = nc.declare_dram_parameter("bias", [P, JT], f32, isOutput=False)
    outp = nc.declare_dram_parameter("outp", [P, JT * RPC], bf, isOutput=True)

    with TileContext(nc) as tc:
        with (
            tc.tile_pool(name="ip", bufs=1) as ip,
            tc.tile_pool(name="pp", bufs=1, space="PSUM") as pp,
            tc.tile_pool(name="op", bufs=1) as op,
        ):
            # junk-warm tile needs no DMA: memset, then matmuls right after
            # the preamble (nonzero data - zeros don't lift the HAM gate).
            wm_t = ip.tile([P, P + JF], bf, name="wm", tag="wm")
            nc.gpsimd.memset(wm_t[:], 1.0)

            hd_t = ip.tile([P, P + RPC], bf, name="hd", tag="hd")
            wc0r_t = ip.tile([P, (KT - 1) * P], bf, name="wc0r", tag="wc0r")
            xsp_t = ip.tile([P, (KT - 1) * RPC], bf, name="xsp", tag="xsp")
            wcp_t = ip.tile([P, (JT - 1) * E], bf, name="wcp", tag="wcp")
            bias_t = ip.tile([P, JT], f32, name="bias", tag="bias")

            def xs_chunk(eng, a, b):   # X slabs a..b-1 (1-based slabs)
                eng.dma_start(
                    out=xsp_t[:, (a - 1) * RPC:(b - 1) * RPC],
                    in_=xsp[:, (a - 1) * RPC:(b - 1) * RPC],
                )

            def w_chunk(eng, a, b):    # W banks a..b-1 (1-based banks)
                eng.dma_start(
                    out=wcp_t[:, (a - 1) * E:(b - 1) * E],
                    in_=wcp[:, (a - 1) * E:(b - 1) * E],
                )

            # Queue split tuned from trace timings (~180-240 B/ns per queue
            # when both stream, ~1.4us DGE start + 0.9us completion-sem
            # latency, ~0.6us sequencer time per dispatch): sync carries the
            # fast-start head, bank0's W remainder and ALL of X (the X
            # arrival cadence gates the bank0/1 sweep); scalar delivers W1-3
            # (needed as banks retire) and the late banks.
            nc.sync.dma_start(out=hd_t[:], in_=hd[:, :])
            xs_chunk(nc.sync, 1, 3)
            xs_chunk(nc.sync, 3, 4)
            xs_chunk(nc.sync, 4, 6)
            xs_chunk(nc.sync, 6, 8)
            w_chunk(nc.sync, 4, 6)
            nc.scalar.dma_start(out=wc0r_t[:], in_=wc0r[:, :])
            nc.scalar.dma_start(out=bias_t[:], in_=bias[:, :])
            w_chunk(nc.scalar, 1, 2)
            w_chunk(nc.scalar, 2, 3)
            w_chunk(nc.scalar, 3, 4)
            w_chunk(nc.scalar, 6, 8)

            ps = [
                pp.tile([P, RPC], f32, name=f"ps{j}", tag=f"ps{j}")
                for j in range(JT)
            ]

            def junk(i, f=JF2):
                nc.tensor.matmul(
                    ps[i % JT][:, 0:f],
                    wm_t[:, 0:P],
                    wm_t[:, P:P + f],
                    start=True,
                    stop=True,
                )

            # PE warm-up on nonzero bf16 data, starting right after the
            # preamble so the HAM clock gate / p-state ramp is underway
            # before the first real matmul.
            for i in range(N_WARM):
                junk(i, JF)

            def lhsT(j, k):
                if j == 0:
                    if k == 0:
                        return hd_t[:, 0:P]
                    return wc0r_t[:, (k - 1) * P:k * P]
                return wcp_t[:, (j - 1) * E + k * P:(j - 1) * E + (k + 1) * P]

            def rhs(k):
                if k == 0:
                    return hd_t[:, P:P + RPC]
                return xsp_t[:, (k - 1) * RPC:k * RPC]

            ob = op.tile([P, JT * RPC], bf, name="ob", tag="ob")

            def mm(j, k):
                nc.tensor.matmul(
                    ps[j], lhsT(j, k), rhs(k),
                    start=(k == 0), stop=(k == KT - 1),
                )

            def evict(j):
                o = ob[:, j * RPC:(j + 1) * RPC]
                if j < JT - 1:
                    nc.vector.tensor_scalar_add(o, ps[j], bias_t[:, j:j + 1])
                    nc.sync.dma_start(
                        out=outp[:, j * RPC:(j + 1) * RPC], in_=o
                    )
                else:
                    # final eviction in halves on TWO engines (DVE + Act) so
                    # the halves and their output DMAs (one per queue) all
                    # overlap, shrinking the post-matmul tail.
                    hh = RPC // 2
                    nc.vector.tensor_scalar_add(
                        o[:, 0:hh], ps[j][:, 0:hh], bias_t[:, j:j + 1]
                    )
                    nc.sync.dma_start(
                        out=outp[:, j * RPC:j * RPC + hh], in_=o[:, 0:hh]
                    )
                    nc.scalar.activation(
                        o[:, hh:RPC],
                        ps[j][:, hh:RPC],
                        mybir.ActivationFunctionType.Identity,
                        bias=bias_t[:, j:j + 1],
                    )
                    nc.scalar.dma_start(
                        out=outp[:, j * RPC + hh:(j + 1) * RPC],
                        in_=o[:, hh:RPC],
                    )

            # X-gated phase: banks 0 and 1 interleaved, k groups following
            # the X chunk arrival order, with junk fillers absorbing stream
            # jitter so the PE p-state ramp never resets.  Banks 2-7 follow
            # back-to-back (compute-bound), each retiring with an eviction +
            # output DMA that overlap the remaining compute.
            fb = [2, 3, 4, 5, 6, 7]   # filler-target banks (not started yet)

            def fill(n):
                for _ in range(n):
                    junk(fb[fill.i % len(fb)])
                    fill.i += 1
            fill.i = 0

            mm(0, 0)
            fill(2)
            mm(0, 1)
            mm(0, 2)
            fill(1)
            mm(0, 3)
            fill(1)
            for k in (0, 1, 2, 3):
                mm(1, k)
            fill(1)
            mm(0, 4)
            mm(0, 5)
            mm(1, 4)
            mm(1, 5)
            fill(1)
            mm(0, 6)
            mm(0, 7)
            mm(1, 6)
            mm(1, 7)
            evict(0)
            evict(1)
            for j in range(2, JT):
                for k in range(KT):
                    mm(j, k)
                evict(j)
    nc.compile()
    return nc


def _get_nc():
    if "nc" not in _NC_CACHE:
        _NC_CACHE["nc"] = _build()
    return _NC_CACHE["nc"]


def _prep_in_maps(V, Wv, bv, Wo, bo, lq):
    Wv64 = np.asarray(Wv, np.float64)
    Wo64 = np.asarray(Wo, np.float64)
    bv64 = np.asarray(bv, np.float64)
    bo64 = np.asarray(bo, np.float64)

    # Fold per-head V-projection + output projection + attention mass (== Lq).
    Wo_r = Wo64.reshape(E, H, HD)                       # [n, h, b]
    W_eff = lq * np.einsum("ba,nhb->han", Wv64, Wo_r, optimize=True)
    W_eff = W_eff.reshape(E, E).astype(np.float32)      # [k, n]
    b_eff = (lq * np.einsum("nhb,b->n", Wo_r, bv64) + bo64).astype(np.float32)

    # wc_all[p, j*E + k*P + c] = W_eff[k*P + p, j*P + c]  (lhsT blocks)
    wc_all = np.ascontiguousarray(
        W_eff.reshape(KT, P, JT, P).transpose(1, 2, 0, 3).reshape(P, JT * E)
    ).astype(BF16)
    bias_blk = np.ascontiguousarray(b_eff.reshape(JT, P).T)   # [p, j] f32

    X = np.asarray(V, dtype=np.float32).reshape(ROWS, E).astype(BF16)
    wc0r = np.ascontiguousarray(wc_all[:, P:E])
    wcp = np.ascontiguousarray(wc_all[:, E:])
    in_maps = []
    for i in range(N_CORES):
        xsT = np.ascontiguousarray(X[i * RPC:(i + 1) * RPC, :].T)  # [E, RPC]
        hd_i = np.empty((P, P + RPC), BF16)
        hd_i[:, :P] = wc_all[:, :P]
        hd_i[:, P:] = xsT[0:P, :]
        xsp_i = np.ascontiguousarray(
            xsT.reshape(KT, P, RPC)[1:].transpose(1, 0, 2).reshape(P, (KT - 1) * RPC)
        )
        in_maps.append(
            {"hd": hd_i, "wc0r": wc0r, "xsp": xsp_i, "wcp": wcp, "bias": bias_blk}
        )
    return in_maps


def kernel(Q, K, V, Wq, bq, Wk, bk, Wv, bv, Wo, bo, **_unused):
    global LAST_RESULTS
    n, L, e = np.asarray(V).shape
    lq = float(np.asarray(Q).shape[1])
    in_maps = _prep_in_maps(V, Wv, bv, Wo, bo, lq)
    nc = _get_nc()
    LAST_RESULTS = run_bass_kernel_spmd(nc, in_maps, list(range(N_CORES)))
    parts = []
    for i in range(N_CORES):
        outp = LAST_RESULTS.results[i]["outp"]          # [P, JT*RPC] bf16
        oT = outp.reshape(P, JT, RPC).transpose(1, 0, 2).reshape(E, RPC)
        parts.append(np.ascontiguousarray(oT.T).astype(np.float32))
    out = np.concatenate(parts, axis=0)
    return np.ascontiguousarray(out).reshape(n, L, E)


# revision 16
# speedup vs baseline: 1.0532x; 1.0532x over previous
"""MultiHeadAttention kernel for 8x TRN2 NeuronCores.

The reference module's einsum reduces the attention tensor over BOTH the
query and key axes (attn_mass = sum_{q,k} softmax(logits)_k), and softmax
rows sum to 1, so attn_mass == Lq exactly for every (batch, head).  The
whole computation collapses to a single dense GEMM after folding the
(block-diagonal) per-head V-projection into the output projection:

    out = V_flat @ W_eff + b_eff          (4096 x 1024) @ (1024 x 1024)
    W_eff[h*hd+a, n] = Lq * sum_b Wv[b, a] * Wo[n, h*hd+b]
    b_eff[n]         = Lq * sum_{h,b} Wo[n, h*hd+b] * bv[b] + bo[n]

Row-sharded across 8 cores (512 rows each), computed TRANSPOSED so the
bias is a per-partition scalar folded into the PSUM eviction.  All
operands stream as bf16 (tolerance is 2e-2; bf16 end-to-end lands at
~2.6e-3), which halves HBM traffic vs fp32 and runs the PE at 1 row/cycle.

Schedule (per core), driven by trace analysis:
  * every HWDGE queue has ~1.4us of DGE start latency + 0.9us completion
    semaphore propagation, and each dma_start burns ~0.65us of the issuing
    engine's sequencer -- so the stream is a handful of large DMAs spread
    over THREE queues (sync / vector / scalar) for dispatch parallelism;
  * a tiny fast-start DMA [W0k0 | X0] (160 KB) leads the sync queue and
    unblocks the first real matmul ~4us in; bf16 junk matmuls on memset
    data keep the PE busy (HAM clock gate + p-state ramp) until then;
  * X is fronted across sync+vector queues (bank 0's k-sweep is gated by
    X arrival), W banks 1-7 follow; banks retire in order afterwards,
    purely compute-bound, so evictions + output DMAs overlap compute;
  * the last bank's eviction is split so its output DMA overlaps the
    second half's tensor_scalar_add.
"""

import numpy as np
import ml_dtypes

import concourse.bass as bass
import concourse.bacc as bacc
import concourse.mybir as mybir
from concourse.tile import TileContext
from concourse.bass_utils import run_bass_kernel_spmd

N_CORES = 8
E = 1024            # embed dim == d_model
H, HD = 16, 64      # heads, head dim
ROWS = 4096         # N * L = 2 * 2048
RPC = ROWS // N_CORES   # rows per core = 512
P = 128             # SBUF partitions
KT = E // P         # 8 contraction slabs
JT = E // P         # 8 output-column banks

# -- tuning knobs ------------------------------------------------------
N_WARM = 7          # junk matmuls before the first real matmul
JF = 512            # junk matmul free dim
JF2 = 256           # filler junk free dim (inside the X-gated phase)

BF16 = ml_dtypes.bfloat16

_NC_CACHE = {}
LAST_RESULTS = None  # BassKernelResults of the most recent device run


def _build():
    f32 = mybir.dt.float32
    bf = mybir.dt.bfloat16
    nc = bacc.Bacc(None, target_bir_lowering=False)

    # hd = [W0k0 | X0] fast-start block; wc0r = W0 k=1..7; xsp = X slabs
    # 1..7 packed; wcp = W banks 1..7 packed; bias per-bank per-partition.
    hd = nc.declare_dram_parameter("hd", [P, P + RPC], bf, isOutput=False)
    wc0r = nc.declare_dram_parameter("wc0r", [P, (KT - 1) * P], bf, isOutput=False)
    xsp = nc.declare_dram_parameter("xsp", [P, (KT - 1) * RPC], bf, isOutput=False)
    wcp = nc.declare_dram_parameter("wcp", [P, (JT - 1) * E], bf, isOutput=False)
    bias = nc.declare_dram_parameter("bias", [P, JT], f32, isOutput=False)
    outp = nc.declare_dram_parameter("outp", [P, JT * RPC], bf, isOutput=True)

    with TileContext(nc) as tc:
        with (
            tc.tile_pool(name="ip", bufs=1) as ip,
            tc.tile_pool(name="pp", bufs=1, space="PSUM") as pp,
            tc.tile_pool(name="op", bufs=1) as op,
        ):
            # junk-warm tile needs no DMA: memset, then matmuls right after
            # the preamble (nonzero data - zeros don't lift the HAM gate).
            wm_t = ip.tile([P, P + JF], bf, name="wm", tag="wm")
            nc.gpsimd.memset(wm_t[:], 1.0)

            hd_t = ip.tile([P, P + RPC], bf, name="hd", tag="hd")
            wc0r_t = ip.tile([P, (KT - 1) * P], bf, name="wc0r", tag="wc0r")
            xsp_t = ip.tile([P, (KT - 1) * RPC], bf, name="xsp", tag="xsp")
            wcp_t = ip.tile([P, (JT - 1) * E], bf, name="wcp", tag="wcp")
            bias_t = ip.tile([P, JT], f32, name="bias", tag="bias")

            def xs_chunk(eng, a, b):   # X slabs a..b-1 (1-based slabs)
                eng.dma_start(
                    out=xsp_t[:, (a - 1) * RPC:(b - 1) * RPC],
                    in_=xsp[:, (a - 1) * RPC:(b - 1) * RPC],
                )

            def w_chunk(eng, a, b):    # W banks a..b-1 (1-based banks)
                eng.dma_start(
                    out=wcp_t[:, (a - 1) * E:(b - 1) * E],
                    in_=wcp[:, (a - 1) * E:(b - 1) * E],
                )

            # Queue split tuned from trace timings (~180-240 B/ns per queue
            # when both stream, ~1.4us DGE start + 0.9us completion-sem
            # latency, ~0.6us sequencer time per dispatch): sync carries the
            # fast-start head, bank0's W remainder and ALL of X (the X
            # arrival cadence gates the bank0/1 sweep); scalar delivers W1-3
            # (needed as banks retire) and the late banks.
            nc.sync.dma_start(out=hd_t[:], in_=hd[:, :])
            xs_chunk(nc.sync, 1, 3)
            xs_chunk(nc.sync, 3, 4)
            xs_chunk(nc.sync, 4, 6)
            xs_chunk(nc.sync, 6, 8)
            w_chunk(nc.sync, 4, 6)
            nc.scalar.dma_start(out=wc0r_t[:], in_=wc0r[:, :])
            nc.scalar.dma_start(out=bias_t[:], in_=bias[:, :])
            w_chunk(nc.scalar, 1, 2)
            w_chunk(nc.scalar, 2, 3)
            w_chunk(nc.scalar, 3, 4)
            w_chunk(nc.scalar, 6, 8)

            ps = [
                pp.tile([P, RPC], f32, name=f"ps{j}", tag=f"ps{j}")
                for j in range(JT)
            ]

            def junk(i, f=JF2):
                nc.tensor.matmul(
                    ps[i % JT][:, 0:f],
                    wm_t[:, 0:P],
                    wm_t[:, P:P + f],
                    start=True,
                    stop=True,
                )

            # PE warm-up on nonzero bf16 data, starting right after the
            # preamble so the HAM clock gate / p-state ramp is underway
            # before the first real matmul.
            for i in range(N_WARM):
                junk(i, JF)

            def lhsT(j, k):
                if j == 0:
                    if k == 0:
                        return hd_t[:, 0:P]
                    return wc0r_t[:, (k - 1) * P:k * P]
                return wcp_t[:, (j - 1) * E + k * P:(j - 1) * E + (k + 1) * P]

            def rhs(k):
                if k == 0:
                    return hd_t[:, P:P + RPC]
                return xsp_t[:, (k - 1) * RPC:k * RPC]

            ob = op.tile([P, JT * RPC], bf, name="ob", tag="ob")

            def mm(j, k):
                nc.tensor.matmul(
                    ps[j], lhsT(j, k), rhs(k),
                    start=(k == 0), stop=(k == KT - 1),
                )

            def evict(j):
                o = ob[:, j * RPC:(j + 1) * RPC]
                if j < JT - 1:
                    nc.vector.tensor_scalar_add(o, ps[j], bias_t[:, j:j + 1])
                    nc.sync.dma_start(
                        out=outp[:, j * RPC:(j + 1) * RPC], in_=o
                    )
                else:
                    # final eviction in halves on TWO engines (DVE + Act) so
                    # the halves and their output DMAs (one per queue) all
                    # overlap, shrinking the post-matmul tail.
                    hh = RPC // 2
                    nc.vector.tensor_scalar_add(
                        o[:, 0:hh], ps[j][:, 0:hh], bias_t[:, j:j + 1]
                    )
                    nc.sync.dma_start(
                        out=outp[:, j * RPC:j * RPC + hh], in_=o[:, 0:hh]
                    )
                    nc.scalar.activation(
                        o[:, hh:RPC],
                        ps[j][:, hh:RPC],
                        mybir.ActivationFunctionType.Identity,
                        bias=bias_t[:, j:j + 1],
                    )
                    nc.scalar.dma_start(
                        out=outp[:, j * RPC + hh:(j + 1) * RPC],
                        in_=o[:, hh:RPC],
                    )

            # X-gated phase: banks 0 and 1 interleaved, k groups following
            # the X chunk arrival order, with junk fillers absorbing stream
            # jitter so the PE p-state ramp never resets.  Banks 2-7 follow
            # back-to-back (compute-bound), each retiring with an eviction +
            # output DMA that overlap the remaining compute.
            fb = [2, 3, 4, 5, 6, 7]   # filler-target banks (not started yet)

            def fill(n):
                for _ in range(n):
                    junk(fb[fill.i % len(fb)])
                    fill.i += 1
            fill.i = 0

            mm(0, 0)
            fill(3)
            mm(0, 1)
            fill(1)
            mm(0, 2)
            fill(1)
            mm(0, 3)
            fill(1)
            mm(1, 0)
            mm(1, 1)
            fill(1)
            mm(1, 2)
            mm(1, 3)
            fill(1)
            mm(0, 4)
            mm(0, 5)
            fill(1)
            mm(1, 4)
            mm(1, 5)
            fill(1)
            mm(0, 6)
            mm(0, 7)
            mm(1, 6)
            mm(1, 7)
            evict(0)
            evict(1)
            for j in range(2, JT):
                for k in range(KT):
                    mm(j, k)
                evict(j)
    nc.compile()
    return nc


def _get_nc():
    if "nc" not in _NC_CACHE:
        _NC_CACHE["nc"] = _build()
    return _NC_CACHE["nc"]


def _prep_in_maps(V, Wv, bv, Wo, bo, lq):
    Wv64 = np.asarray(Wv, np.float64)
    Wo64 = np.asarray(Wo, np.float64)
    bv64 = np.asarray(bv, np.float64)
    bo64 = np.asarray(bo, np.float64)

    # Fold per-head V-projection + output projection + attention mass (== Lq).
    Wo_r = Wo64.reshape(E, H, HD)                       # [n, h, b]
    W_eff = lq * np.einsum("ba,nhb->han", Wv64, Wo_r, optimize=True)
    W_eff = W_eff.reshape(E, E).astype(np.float32)      # [k, n]
    b_eff = (lq * np.einsum("nhb,b->n", Wo_r, bv64) + bo64).astype(np.float32)

    # wc_all[p, j*E + k*P + c] = W_eff[k*P + p, j*P + c]  (lhsT blocks)
    wc_all = np.ascontiguousarray(
        W_eff.reshape(KT, P, JT, P).transpose(1, 2, 0, 3).reshape(P, JT * E)
    ).astype(BF16)
    bias_blk = np.ascontiguousarray(b_eff.reshape(JT, P).T)   # [p, j] f32

    X = np.asarray(V, dtype=np.float32).reshape(ROWS, E).astype(BF16)
    wc0r = np.ascontiguousarray(wc_all[:, P:E])
    wcp = np.ascontiguousarray(wc_all[:, E:])
    in_maps = []
    for i in range(N_CORES):
        xsT = np.ascontiguousarray(X[i * RPC:(i + 1) * RPC, :].T)  # [E, RPC]
        hd_i = np.empty((P, P + RPC), BF16)
        hd_i[:, :P] = wc_all[:, :P]
        hd_i[:, P:] = xsT[0:P, :]
        xsp_i = np.ascontiguousarray(
            xsT.reshape(KT, P, RPC)[1:].transpose(1, 0, 2).reshape(P, (KT - 1) * RPC)
        )
        in_maps.append(
            {"hd": hd_i, "wc0r": wc0r, "xsp": xsp_i, "wcp": wcp, "bias": bias_blk}
        )
    return in_maps


def kernel(Q, K, V, Wq, bq, Wk, bk, Wv, bv, Wo, bo, **_unused):
    global LAST_RESULTS
    n, L, e = np.asarray(V).shape
    lq = float(np.asarray(Q).shape[1])
    in_maps = _prep_in_maps(V, Wv, bv, Wo, bo, lq)
    nc = _get_nc()
    LAST_RESULTS = run_bass_kernel_spmd(nc, in_maps, list(range(N_CORES)))
    parts = []
    for i in range(N_CORES):
        outp = LAST_RESULTS.results[i]["outp"]          # [P, JT*RPC] bf16
        oT = outp.reshape(P, JT, RPC).transpose(1, 0, 2).reshape(E, RPC)
        parts.append(np.ascontiguousarray(oT.T).astype(np.float32))
    out = np.concatenate(parts, axis=0)
    return np.ascontiguousarray(out).reshape(n, L, E)


# revision 20
# speedup vs baseline: 1.0776x; 1.0231x over previous
"""MultiHeadAttention kernel for 8x TRN2 NeuronCores.

The reference module's einsum reduces the attention tensor over BOTH the
query and key axes (attn_mass = sum_{q,k} softmax(logits)_k), and softmax
rows sum to 1, so attn_mass == Lq exactly for every (batch, head).  The
whole computation collapses to a single dense GEMM after folding the
(block-diagonal) per-head V-projection into the output projection:

    out = V_flat @ W_eff + b_eff          (4096 x 1024) @ (1024 x 1024)
    W_eff[h*hd+a, n] = Lq * sum_b Wv[b, a] * Wo[n, h*hd+b]
    b_eff[n]         = Lq * sum_{h,b} Wo[n, h*hd+b] * bv[b] + bo[n]

Row-sharded across 8 cores (512 rows each), computed TRANSPOSED so the
bias is a per-partition scalar folded into the PSUM eviction.  All
operands stream as bf16 (tolerance is 2e-2; bf16 end-to-end lands at
~2.6e-3), which halves HBM traffic vs fp32 and runs the PE at 1 row/cycle.

Schedule (per core), driven by trace analysis:
  * every HWDGE queue has ~1.4us of DGE start latency + 0.9us completion
    semaphore propagation, and each dma_start burns ~0.65us of the issuing
    engine's sequencer -- so the stream is a handful of large DMAs spread
    over THREE queues (sync / vector / scalar) for dispatch parallelism;
  * a tiny fast-start DMA [W0k0 | X0] (160 KB) leads the sync queue and
    unblocks the first real matmul ~4us in; bf16 junk matmuls on memset
    data keep the PE busy (HAM clock gate + p-state ramp) until then;
  * X is fronted across sync+vector queues (bank 0's k-sweep is gated by
    X arrival), W banks 1-7 follow; banks retire in order afterwards,
    purely compute-bound, so evictions + output DMAs overlap compute;
  * the last bank's eviction is split so its output DMA overlaps the
    second half's tensor_scalar_add.
"""

import numpy as np
import ml_dtypes

import concourse.bass as bass
import concourse.bacc as bacc
import concourse.mybir as mybir
from concourse.tile import TileContext
from concourse.bass_utils import run_bass_kernel_spmd

N_CORES = 8
E = 1024            # embed dim == d_model
H, HD = 16, 64      # heads, head dim
ROWS = 4096         # N * L = 2 * 2048
RPC = ROWS // N_CORES   # rows per core = 512
P = 128             # SBUF partitions
KT = E // P         # 8 contraction slabs
JT = E // P         # 8 output-column banks

# -- tuning knobs ------------------------------------------------------
N_WARM = 7          # junk matmuls before the first real matmul
JF = 512            # junk matmul free dim
JF2 = 256           # filler junk free dim (inside the X-gated phase)

BF16 = ml_dtypes.bfloat16

_NC_CACHE = {}
LAST_RESULTS = None  # BassKernelResults of the most recent device run


def _build():
    f32 = mybir.dt.float32
    bf = mybir.dt.bfloat16
    nc = bacc.Bacc(None, target_bir_lowering=False)

    # hd = [W0k0 | X0] fast-start block; wc0r = W0 k=1..7; xsp = X slabs
    # 1..7 packed; wcp = W banks 1..7 packed; bias per-bank per-partition.
    hd = nc.declare_dram_parameter("hd", [P, P + RPC], bf, isOutput=False)
    wc0r = nc.declare_dram_parameter("wc0r", [P, (KT - 1) * P], bf, isOutput=False)
    xsp = nc.declare_dram_parameter("xsp", [P, (KT - 1) * RPC], bf, isOutput=False)
    wcp = nc.declare_dram_parameter("wcp", [P, (JT - 1) * E], bf, isOutput=False)
    bias = nc.declare_dram_parameter("bias", [P, JT], f32, isOutput=False)
    outp = nc.declare_dram_parameter("outp", [P, JT * RPC], bf, isOutput=True)

    with TileContext(nc) as tc:
        with (
            tc.tile_pool(name="ip", bufs=1) as ip,
            tc.tile_pool(name="pp", bufs=1, space="PSUM") as pp,
            tc.tile_pool(name="op", bufs=1) as op,
        ):
            # junk-warm tile needs no DMA: memset, then matmuls right after
            # the preamble (nonzero data - zeros don't lift the HAM gate).
            wm_t = ip.tile([P, P + JF], bf, name="wm", tag="wm")
            nc.gpsimd.memset(wm_t[:], 1.0)

            hd_t = ip.tile([P, P + RPC], bf, name="hd", tag="hd")
            wc0r_t = ip.tile([P, (KT - 1) * P], bf, name="wc0r", tag="wc0r")
            xsp_t = ip.tile([P, (KT - 1) * RPC], bf, name="xsp", tag="xsp")
            wcp_t = ip.tile([P, (JT - 1) * E], bf, name="wcp", tag="wcp")
            bias_t = ip.tile([P, JT], f32, name="bias", tag="bias")

            def xs_chunk(eng, a, b):   # X slabs a..b-1 (1-based slabs)
                eng.dma_start(
                    out=xsp_t[:, (a - 1) * RPC:(b - 1) * RPC],
                    in_=xsp[:, (a - 1) * RPC:(b - 1) * RPC],
                )

            def w_chunk(eng, a, b):    # W banks a..b-1 (1-based banks)
                eng.dma_start(
                    out=wcp_t[:, (a - 1) * E:(b - 1) * E],
                    in_=wcp[:, (a - 1) * E:(b - 1) * E],
                )

            # Queue split tuned from trace timings (~180-240 B/ns per queue
            # when both stream, ~1.4us DGE start + 0.9us completion-sem
            # latency, ~0.6us sequencer time per dispatch): sync carries the
            # fast-start head, bank0's W remainder and ALL of X (the X
            # arrival cadence gates the bank0/1 sweep); scalar delivers W1-3
            # (needed as banks retire) and the late banks.
            nc.sync.dma_start(out=hd_t[:], in_=hd[:, :])
            xs_chunk(nc.sync, 1, 3)
            xs_chunk(nc.sync, 4, 6)
            xs_chunk(nc.sync, 6, 8)
            w_chunk(nc.sync, 4, 6)
            nc.scalar.dma_start(out=wc0r_t[:], in_=wc0r[:, :])
            nc.scalar.dma_start(out=bias_t[:], in_=bias[:, :])
            xs_chunk(nc.scalar, 3, 4)
            w_chunk(nc.scalar, 1, 2)
            w_chunk(nc.scalar, 2, 3)
            w_chunk(nc.scalar, 3, 4)
            w_chunk(nc.scalar, 6, 8)

            ps = [
                pp.tile([P, RPC], f32, name=f"ps{j}", tag=f"ps{j}")
                for j in range(JT)
            ]

            def junk(i, f=JF2):
                nc.tensor.matmul(
                    ps[i % JT][:, 0:f],
                    wm_t[:, 0:P],
                    wm_t[:, P:P + f],
                    start=True,
                    stop=True,
                )

            # PE warm-up on nonzero bf16 data, starting right after the
            # preamble so the HAM clock gate / p-state ramp is underway
            # before the first real matmul.
            for i in range(N_WARM):
                junk(i, JF)

            def lhsT(j, k):
                if j == 0:
                    if k == 0:
                        return hd_t[:, 0:P]
                    return wc0r_t[:, (k - 1) * P:k * P]
                return wcp_t[:, (j - 1) * E + k * P:(j - 1) * E + (k + 1) * P]

            def rhs(k):
                if k == 0:
                    return hd_t[:, P:P + RPC]
                return xsp_t[:, (k - 1) * RPC:k * RPC]

            ob = op.tile([P, JT * RPC], bf, name="ob", tag="ob")

            def mm(j, k):
                nc.tensor.matmul(
                    ps[j], lhsT(j, k), rhs(k),
                    start=(k == 0), stop=(k == KT - 1),
                )

            def evict(j):
                # alternate output queues so neither engine's dispatch train
                # backs up behind the other banks' output DMAs
                o = ob[:, j * RPC:(j + 1) * RPC]
                eng = nc.sync if j % 2 == 0 else nc.scalar
                nc.vector.tensor_scalar_add(o, ps[j], bias_t[:, j:j + 1])
                eng.dma_start(out=outp[:, j * RPC:(j + 1) * RPC], in_=o)

            def evict7_half(h):
                # bank 7 accumulates in row-halves; half 0's eviction + DMA
                # overlap half 1's matmuls.  The very last half is evicted in
                # quarters on TWO engines (DVE + Act) with output DMAs on
                # both queues, minimizing the post-matmul serial tail.
                hh = RPC // 2
                j = JT - 1
                o = ob[:, j * RPC:(j + 1) * RPC]
                if h == 0:
                    nc.vector.tensor_scalar_add(
                        o[:, 0:hh], ps[j][:, 0:hh], bias_t[:, j:j + 1]
                    )
                    nc.sync.dma_start(
                        out=outp[:, j * RPC:j * RPC + hh], in_=o[:, 0:hh]
                    )
                else:
                    qq = hh // 2
                    nc.vector.tensor_scalar_add(
                        o[:, hh:hh + qq], ps[j][:, hh:hh + qq], bias_t[:, j:j + 1]
                    )
                    nc.sync.dma_start(
                        out=outp[:, j * RPC + hh:j * RPC + hh + qq],
                        in_=o[:, hh:hh + qq],
                    )
                    nc.scalar.activation(
                        o[:, hh + qq:RPC],
                        ps[j][:, hh + qq:RPC],
                        mybir.ActivationFunctionType.Identity,
                        bias=bias_t[:, j:j + 1],
                    )
                    nc.scalar.dma_start(
                        out=outp[:, j * RPC + hh + qq:(j + 1) * RPC],
                        in_=o[:, hh + qq:RPC],
                    )

            # X-gated phase: banks 0 and 1 interleaved, k groups following
            # the X chunk arrival order, with junk fillers absorbing stream
            # jitter so the PE p-state ramp never resets.  Banks 2-7 follow
            # back-to-back (compute-bound), each retiring with an eviction +
            # output DMA that overlap the remaining compute.
            fb = [2, 3, 4, 5, 6, 7]   # filler-target banks (not started yet)

            def fill(n):
                for _ in range(n):
                    junk(fb[fill.i % len(fb)])
                    fill.i += 1
            fill.i = 0

            mm(0, 0)
            fill(4)
            mm(0, 1)
            fill(1)
            mm(0, 2)
            fill(1)
            mm(0, 3)
            fill(1)
            mm(1, 0)
            mm(1, 1)
            fill(1)
            mm(1, 2)
            mm(1, 3)
            fill(1)
            mm(0, 4)
            mm(0, 5)
            fill(1)
            mm(1, 4)
            mm(1, 5)
            fill(1)
            mm(0, 6)
            mm(0, 7)
            mm(1, 6)
            mm(1, 7)
            evict(0)
            evict(1)
            for j in range(2, JT - 1):
                for k in range(KT):
                    mm(j, k)
                evict(j)

            # bank 7 in row-halves (see evict7_half)
            hh = RPC // 2

            def mm7(h, k):
                r = rhs(k)
                nc.tensor.matmul(
                    ps[JT - 1][:, h * hh:(h + 1) * hh],
                    lhsT(JT - 1, k),
                    r[:, h * hh:(h + 1) * hh],
                    start=(k == 0),
                    stop=(k == KT - 1),
                )

            for h in range(2):
                for k in range(KT):
                    mm7(h, k)
                evict7_half(h)
    nc.compile()
    return nc


def _get_nc():
    if "nc" not in _NC_CACHE:
        _NC_CACHE["nc"] = _build()
    return _NC_CACHE["nc"]


def _prep_in_maps(V, Wv, bv, Wo, bo, lq):
    Wv64 = np.asarray(Wv, np.float64)
    Wo64 = np.asarray(Wo, np.float64)
    bv64 = np.asarray(bv, np.float64)
    bo64 = np.asarray(bo, np.float64)

    # Fold per-head V-projection + output projection + attention mass (== Lq).
    Wo_r = Wo64.reshape(E, H, HD)                       # [n, h, b]
    W_eff = lq * np.einsum("ba,nhb->han", Wv64, Wo_r, optimize=True)
    W_eff = W_eff.reshape(E, E).astype(np.float32)      # [k, n]
    b_eff = (lq * np.einsum("nhb,b->n", Wo_r, bv64) + bo64).astype(np.float32)

    # wc_all[p, j*E + k*P + c] = W_eff[k*P + p, j*P + c]  (lhsT blocks)
    wc_all = np.ascontiguousarray(
        W_eff.reshape(KT, P, JT, P).transpose(1, 2, 0, 3).reshape(P, JT * E)
    ).astype(BF16)
    bias_blk = np.ascontiguousarray(b_eff.reshape(JT, P).T)   # [p, j] f32

    X = np.asarray(V, dtype=np.float32).reshape(ROWS, E).astype(BF16)
    wc0r = np.ascontiguousarray(wc_all[:, P:E])
    wcp = np.ascontiguousarray(wc_all[:, E:])
    in_maps = []
    for i in range(N_CORES):
        xsT = np.ascontiguousarray(X[i * RPC:(i + 1) * RPC, :].T)  # [E, RPC]
        hd_i = np.empty((P, P + RPC), BF16)
        hd_i[:, :P] = wc_all[:, :P]
        hd_i[:, P:] = xsT[0:P, :]
        xsp_i = np.ascontiguousarray(
            xsT.reshape(KT, P, RPC)[1:].transpose(1, 0, 2).reshape(P, (KT - 1) * RPC)
        )
        in_maps.append(
            {"hd": hd_i, "wc0r": wc0r, "xsp": xsp_i, "wcp": wcp, "bias": bias_blk}
        )
    return in_maps


def kernel(Q, K, V, Wq, bq, Wk, bk, Wv, bv, Wo, bo, **_unused):
    global LAST_RESULTS
    n, L, e = np.asarray(V).shape
    lq = float(np.asarray(Q).shape[1])
    in_maps = _prep_in_maps(V, Wv, bv, Wo, bo, lq)
    nc = _get_nc()
    LAST_RESULTS = run_bass_kernel_spmd(nc, in_maps, list(range(N_CORES)))
    parts = []
    for i in range(N_CORES):
        outp = LAST_RESULTS.results[i]["outp"]          # [P, JT*RPC] bf16
        oT = outp.reshape(P, JT, RPC).transpose(1, 0, 2).reshape(E, RPC)
        parts.append(np.ascontiguousarray(oT.T).astype(np.float32))
    out = np.concatenate(parts, axis=0)
    return np.ascontiguousarray(out).reshape(n, L, E)


# revision 23
# speedup vs baseline: 1.1069x; 1.0272x over previous
"""MultiHeadAttention kernel for 8x TRN2 NeuronCores.

The reference module's einsum reduces the attention tensor over BOTH the
query and key axes (attn_mass = sum_{q,k} softmax(logits)_k), and softmax
rows sum to 1, so attn_mass == Lq exactly for every (batch, head).  The
whole computation collapses to a single dense GEMM after folding the
(block-diagonal) per-head V-projection into the output projection:

    out = V_flat @ W_eff + b_eff          (4096 x 1024) @ (1024 x 1024)
    W_eff[h*hd+a, n] = Lq * sum_b Wv[b, a] * Wo[n, h*hd+b]
    b_eff[n]         = Lq * sum_{h,b} Wo[n, h*hd+b] * bv[b] + bo[n]

Row-sharded across 8 cores (512 rows each), computed TRANSPOSED so the
bias is a per-partition scalar folded into the PSUM eviction.  All
operands stream as bf16 (tolerance is 2e-2; bf16 end-to-end lands at
~2.6e-3), which halves HBM traffic vs fp32 and runs the PE at 1 row/cycle.

Schedule (per core), driven by trace analysis:
  * every HWDGE queue has ~1.4us of DGE start latency + 0.9us completion
    semaphore propagation, and each dma_start burns ~0.65us of the issuing
    engine's sequencer -- so the stream is a handful of large DMAs spread
    over THREE queues (sync / vector / scalar) for dispatch parallelism;
  * a tiny fast-start DMA [W0k0 | X0] (160 KB) leads the sync queue and
    unblocks the first real matmul ~4us in; bf16 junk matmuls on memset
    data keep the PE busy (HAM clock gate + p-state ramp) until then;
  * X is fronted across sync+vector queues (bank 0's k-sweep is gated by
    X arrival), W banks 1-7 follow; banks retire in order afterwards,
    purely compute-bound, so evictions + output DMAs overlap compute;
  * the last bank's eviction is split so its output DMA overlaps the
    second half's tensor_scalar_add.
"""

import numpy as np
import ml_dtypes

import concourse.bass as bass
import concourse.bacc as bacc
import concourse.mybir as mybir
from concourse.tile import TileContext
from concourse.bass_utils import run_bass_kernel_spmd

N_CORES = 8
E = 1024            # embed dim == d_model
H, HD = 16, 64      # heads, head dim
ROWS = 4096         # N * L = 2 * 2048
RPC = ROWS // N_CORES   # rows per core = 512
P = 128             # SBUF partitions
KT = E // P         # 8 contraction slabs
JT = E // P         # 8 output-column banks

# -- tuning knobs ------------------------------------------------------
N_WARM = 7          # junk matmuls before the first real matmul
JF = 512            # junk matmul free dim
JF2 = 256           # filler junk free dim (inside the X-gated phase)

BF16 = ml_dtypes.bfloat16

_NC_CACHE = {}
LAST_RESULTS = None  # BassKernelResults of the most recent device run


def _build():
    f32 = mybir.dt.float32
    bf = mybir.dt.bfloat16
    nc = bacc.Bacc(None, target_bir_lowering=False)

    # hd = [W0k0 | X0] fast-start block; wc0r = W0 k=1..7; xsp = X slabs
    # 1..7 packed; wcp = W banks 1..7 packed; bias per-bank per-partition.
    hd = nc.declare_dram_parameter("hd", [P, P + RPC], bf, isOutput=False)
    wc0r = nc.declare_dram_parameter("wc0r", [P, (KT - 1) * P], bf, isOutput=False)
    xsp = nc.declare_dram_parameter("xsp", [P, (KT - 1) * RPC], bf, isOutput=False)
    wcp = nc.declare_dram_parameter("wcp", [P, (JT - 1) * E], bf, isOutput=False)
    bias = nc.declare_dram_parameter("bias", [P, JT], f32, isOutput=False)
    outp = nc.declare_dram_parameter("outp", [P, JT * RPC], bf, isOutput=True)

    with TileContext(nc) as tc:
        with (
            tc.tile_pool(name="ip", bufs=1) as ip,
            tc.tile_pool(name="pp", bufs=1, space="PSUM") as pp,
            tc.tile_pool(name="op", bufs=1) as op,
        ):
            # junk-warm tile needs no DMA: memset, then matmuls right after
            # the preamble (nonzero data - zeros don't lift the HAM gate).
            wm_t = ip.tile([P, P + JF], bf, name="wm", tag="wm")
            nc.gpsimd.memset(wm_t[:], 1.0)

            hd_t = ip.tile([P, P + RPC], bf, name="hd", tag="hd")
            wc0r_t = ip.tile([P, (KT - 1) * P], bf, name="wc0r", tag="wc0r")
            xsp_t = ip.tile([P, (KT - 1) * RPC], bf, name="xsp", tag="xsp")
            wcp_t = ip.tile([P, (JT - 1) * E], bf, name="wcp", tag="wcp")
            bias_t = ip.tile([P, JT], f32, name="bias", tag="bias")

            def xs_chunk(eng, a, b):   # X slabs a..b-1 (1-based slabs)
                eng.dma_start(
                    out=xsp_t[:, (a - 1) * RPC:(b - 1) * RPC],
                    in_=xsp[:, (a - 1) * RPC:(b - 1) * RPC],
                )

            def w_chunk(eng, a, b):    # W banks a..b-1 (1-based banks)
                eng.dma_start(
                    out=wcp_t[:, (a - 1) * E:(b - 1) * E],
                    in_=wcp[:, (a - 1) * E:(b - 1) * E],
                )

            # Queue split tuned from trace timings (~180-240 B/ns per queue
            # when both stream, ~1.4us DGE start + 0.9us completion-sem
            # latency, ~0.6us sequencer time per dispatch): sync carries the
            # fast-start head, bank0's W remainder and ALL of X (the X
            # arrival cadence gates the bank0/1 sweep); scalar delivers W1-3
            # (needed as banks retire) and the late banks.
            nc.sync.dma_start(out=hd_t[:], in_=hd[:, :])
            xs_chunk(nc.sync, 1, 3)
            xs_chunk(nc.sync, 3, 4)
            xs_chunk(nc.sync, 4, 6)
            xs_chunk(nc.sync, 6, 8)
            w_chunk(nc.sync, 4, 6)
            nc.scalar.dma_start(out=wc0r_t[:], in_=wc0r[:, :])
            nc.scalar.dma_start(out=bias_t[:], in_=bias[:, :])
            w_chunk(nc.scalar, 1, 2)
            w_chunk(nc.scalar, 2, 3)
            w_chunk(nc.scalar, 3, 4)
            w_chunk(nc.scalar, 6, 8)

            ps = [
                pp.tile([P, RPC], f32, name=f"ps{j}", tag=f"ps{j}")
                for j in range(JT)
            ]

            def junk(i, f=JF2):
                nc.tensor.matmul(
                    ps[i % JT][:, 0:f],
                    wm_t[:, 0:P],
                    wm_t[:, P:P + f],
                    start=True,
                    stop=True,
                )

            # PE warm-up on nonzero bf16 data, starting right after the
            # preamble so the HAM clock gate / p-state ramp is underway
            # before the first real matmul.
            for i in range(N_WARM):
                junk(i, JF)

            def lhsT(j, k):
                if j == 0:
                    if k == 0:
                        return hd_t[:, 0:P]
                    return wc0r_t[:, (k - 1) * P:k * P]
                return wcp_t[:, (j - 1) * E + k * P:(j - 1) * E + (k + 1) * P]

            def rhs(k):
                if k == 0:
                    return hd_t[:, P:P + RPC]
                return xsp_t[:, (k - 1) * RPC:k * RPC]

            ob = op.tile([P, JT * RPC], bf, name="ob", tag="ob")

            def mm(j, k):
                nc.tensor.matmul(
                    ps[j], lhsT(j, k), rhs(k),
                    start=(k == 0), stop=(k == KT - 1),
                )

            def evict(j):
                # alternate output queues so neither engine's dispatch train
                # backs up behind the other banks' output DMAs
                o = ob[:, j * RPC:(j + 1) * RPC]
                eng = nc.sync if j % 2 == 0 else nc.scalar
                nc.vector.tensor_scalar_add(o, ps[j], bias_t[:, j:j + 1])
                eng.dma_start(out=outp[:, j * RPC:(j + 1) * RPC], in_=o)

            def evict7(j):
                # final eviction in halves on TWO engines (DVE + Act) so
                # the halves and their output DMAs (one per queue) all
                # overlap, shrinking the post-matmul serial tail.
                hh = RPC // 2
                o = ob[:, j * RPC:(j + 1) * RPC]
                nc.vector.tensor_scalar_add(
                    o[:, 0:hh], ps[j][:, 0:hh], bias_t[:, j:j + 1]
                )
                nc.sync.dma_start(
                    out=outp[:, j * RPC:j * RPC + hh], in_=o[:, 0:hh]
                )
                nc.scalar.activation(
                    o[:, hh:RPC],
                    ps[j][:, hh:RPC],
                    mybir.ActivationFunctionType.Identity,
                    bias=bias_t[:, j:j + 1],
                )
                nc.scalar.dma_start(
                    out=outp[:, j * RPC + hh:(j + 1) * RPC],
                    in_=o[:, hh:RPC],
                )

            # X-gated phase: banks 0 and 1 interleaved, k groups following
            # the X chunk arrival order, with junk fillers absorbing stream
            # jitter so the PE p-state ramp never resets.  Banks 2-7 follow
            # back-to-back (compute-bound), each retiring with an eviction +
            # output DMA that overlap the remaining compute.
            fb = [2, 3, 4, 5, 6, 7]   # filler-target banks (not started yet)

            def fill(n):
                for _ in range(n):
                    junk(fb[fill.i % len(fb)])
                    fill.i += 1
            fill.i = 0

            mm(0, 0)
            fill(4)
            mm(0, 1)
            fill(1)
            mm(0, 2)
            fill(1)
            mm(0, 3)
            fill(1)
            mm(1, 0)
            mm(1, 1)
            fill(1)
            mm(1, 2)
            mm(1, 3)
            fill(1)
            mm(0, 4)
            mm(0, 5)
            fill(1)
            mm(1, 4)
            mm(1, 5)
            fill(1)
            mm(0, 6)
            mm(0, 7)
            mm(1, 6)
            mm(1, 7)
            evict(0)
            evict(1)
            for j in range(2, JT):
                for k in range(KT):
                    mm(j, k)
                if j < JT - 1:
                    evict(j)
                else:
                    evict7(j)
    nc.compile()
    return nc


def _get_nc():
    if "nc" not in _NC_CACHE:
        _NC_CACHE["nc"] = _build()
    return _NC_CACHE["nc"]


def _prep_in_maps(V, Wv, bv, Wo, bo, lq):
    Wv64 = np.asarray(Wv, np.float64)
    Wo64 = np.asarray(Wo, np.float64)
    bv64 = np.asarray(bv, np.float64)
    bo64 = np.asarray(bo, np.float64)

    # Fold per-head V-projection + output projection + attention mass (== Lq).
    Wo_r = Wo64.reshape(E, H, HD)                       # [n, h, b]
    W_eff = lq * np.einsum("ba,nhb->han", Wv64, Wo_r, optimize=True)
    W_eff = W_eff.reshape(E, E).astype(np.float32)      # [k, n]
    b_eff = (lq * np.einsum("nhb,b->n", Wo_r, bv64) + bo64).astype(np.float32)

    # wc_all[p, j*E + k*P + c] = W_eff[k*P + p, j*P + c]  (lhsT blocks)
    wc_all = np.ascontiguousarray(
        W_eff.reshape(KT, P, JT, P).transpose(1, 2, 0, 3).reshape(P, JT * E)
    ).astype(BF16)
    bias_blk = np.ascontiguousarray(b_eff.reshape(JT, P).T)   # [p, j] f32

    X = np.asarray(V, dtype=np.float32).reshape(ROWS, E).astype(BF16)
    wc0r = np.ascontiguousarray(wc_all[:, P:E])
    wcp = np.ascontiguousarray(wc_all[:, E:])
    in_maps = []
    for i in range(N_CORES):
        xsT = np.ascontiguousarray(X[i * RPC:(i + 1) * RPC, :].T)  # [E, RPC]
        hd_i = np.empty((P, P + RPC), BF16)
        hd_i[:, :P] = wc_all[:, :P]
        hd_i[:, P:] = xsT[0:P, :]
        xsp_i = np.ascontiguousarray(
            xsT.reshape(KT, P, RPC)[1:].transpose(1, 0, 2).reshape(P, (KT - 1) * RPC)
        )
        in_maps.append(
            {"hd": hd_i, "wc0r": wc0r, "xsp": xsp_i, "wcp": wcp, "bias": bias_blk}
        )
    return in_maps


def kernel(Q, K, V, Wq, bq, Wk, bk, Wv, bv, Wo, bo, **_unused):
    global LAST_RESULTS
    n, L, e = np.asarray(V).shape
    lq = float(np.asarray(Q).shape[1])
    in_maps = _prep_in_maps(V, Wv, bv, Wo, bo, lq)
    nc = _get_nc()
    LAST_RESULTS = run_bass_kernel_spmd(nc, in_maps, list(range(N_CORES)))
    parts = []
    for i in range(N_CORES):
        outp = LAST_RESULTS.results[i]["outp"]          # [P, JT*RPC] bf16
        oT = outp.reshape(P, JT, RPC).transpose(1, 0, 2).reshape(E, RPC)
        parts.append(np.ascontiguousarray(oT.T).astype(np.float32))
    out = np.concatenate(parts, axis=0)
    return np.ascontiguousarray(out).reshape(n, L, E)
